# revision 14
# baseline (speedup 1.0000x reference)
"""L1-distance attention forward on 8 Trainium2 NeuronCores.

c[b,h,s,t] = -1/sqrt(64) * sum_w |q[b,t,h,w] - k[b,s,h,w]|

Full inputs q,k: [2, 512, 8, 64] f32. Output c: [2, 8, 512, 512] f32.

Sharding: the 16 (b,h) pairs are split 2-per-core across 8 cores (pure data
parallel, no collectives). Each core runs an identical single-core program.
The (cheap, [2,512,64]-sized) transposes of q/k are done host-side so the
device program is pure streaming compute.

Per-core DRAM input (pre-arranged on host), one [128, 772] block per head so a
single DMA feeds the whole head:
  ins[h, w + 64g, 0:512]    = q[t, w]          (both g halves identical)
  ins[h, w + 64g, 512+p]    = k[2p + g, w]
  ins[h, s_local, 768+blk]  = -K_s/8 on VectorE rows, 0 on ScalarE rows
                              (K_s = sum_w k[s, w]; s = 128*blk + s_local)

Per-core algorithm (per head), with s-pairs p = (s0=2p, s1=2p+1):
  - abs-diff tiles d2[w + 64g, t] (fp16, [128, 512]) on two engines:
      ScalarE pairs:  activation(Abs, in_=q2, scale=-1, bias=kb[:, p])
                      -> |q - k| directly
      VectorE pairs:  tensor_scalar(q2, kb[:, p], 0.0, subtract, max) @ 2x
                      -> relu(q - k), using |d| = 2 relu(d) - d and
                         sum_w d = Q_t - K_s handled by rank-1 corrections
  - TensorEngine reduces over w and accumulates 64 s-pairs into one
    [128, 512] PSUM block via sliding-window selector lhsT tiles
    (-1/8 for Abs pairs, -2/8 for relu pairs); one extra fp32 matmul per
    block adds Q_t/8 to VectorE rows using q2 itself as rhs
    (sum_partitions q2 = 2 Q_t, lhsT = 1/16 on VectorE columns).
    PSUM rows are then exactly c[s, t] rows, short of -K_s/8.
  - Staging tensor_scalar(psum + ks_col) -> SBUF adds the K_s correction,
    then DMA out (contiguous 256KB blocks).
"""

import os
from contextlib import ExitStack

import numpy as np

import concourse.bacc as bacc
import concourse.bass as bass
import concourse.mybir as mybir
import concourse.tile as tile
from concourse.bass_utils import run_bass_kernel_spmd

F32 = mybir.dt.float32
F16 = mybir.dt.float16

BS, NCTX, NH, W = 2, 512, 8, 64
N_CORES = 8
HPC = (BS * NH) // N_CORES  # heads (b,h pairs) per core = 2
NPAIR = NCTX // 2  # s-pairs per head = 256
NBLK = NCTX // 128  # 128-row output blocks per head = 4
INS_COLS = NCTX + NPAIR + NBLK  # 772

# s-pairs with (p % 16) < DVE_SPLIT go to VectorE (relu path), the rest to
# ScalarE (abs path). VectorE tensor_scalar runs ~2x the ScalarE rate.
DVE_SPLIT = 11


def _is_dve(p):
    return (p % 16) < DVE_SPLIT


_NC_CACHE = None
LAST_RUN = None  # BassKernelResults of the most recent run (for profiling)


def _build_body(tc, c, ins, reps=1):
    nc = tc.nc
    Abs = mybir.ActivationFunctionType.Abs
    AL = mybir.AluOpType
    with ExitStack() as ctx:
        const = ctx.enter_context(tc.tile_pool(name="const", bufs=1))
        prep = ctx.enter_context(tc.tile_pool(name="prep", bufs=2))
        dpool = ctx.enter_context(tc.tile_pool(name="d2", bufs=8))
        ppool = ctx.enter_context(tc.tile_pool(name="acc", bufs=2, space="PSUM"))
        spool = ctx.enter_context(tc.tile_pool(name="stage", bufs=2))

        # Sliding-window selectors: window [:, 126-2p : 254-2p] has its only
        # nonzero entries in column (126-2p)+m for m = 2p+g, summing partition
        # half g into psum row m. selA = -1/8 (abs pairs), selR = -2/8 (relu).
        selA = const.tile([128, 2 * 64 + 126], F16)
        nc.vector.memset(selA[:], 0.0)
        nc.vector.memset(selA[0:64, 126:127], -0.125)
        nc.vector.memset(selA[64:128, 127:128], -0.125)
        selR = const.tile([128, 2 * 64 + 126], F16)
        nc.vector.memset(selR[:], 0.0)
        nc.vector.memset(selR[0:64, 126:127], -0.25)
        nc.vector.memset(selR[64:128, 127:128], -0.25)
        # Q-correction selector: out[m, t] += (1/16) * sum_wg q2[wg, t] = Q_t/8
        # for VectorE rows m only (column zeroed for ScalarE rows).
        selQ = const.tile([128, 128], F32)
        nc.vector.memset(selQ[:], 1.0 / 16.0)
        for m0 in range(0, 128, 32):
            # rows m = 2p+g with p % 16 >= DVE_SPLIT -> cols [m0+2*DVE_SPLIT, m0+32)
            nc.vector.memset(selQ[:, m0 + 2 * DVE_SPLIT : m0 + 32], 0.0)

        for h in [h for _ in range(reps) for h in range(HPC)]:
            hin = prep.tile([128, INS_COLS], F32, tag="hin")
            nc.sync.dma_start(hin[:], ins[h])
            q2 = hin[:, 0:NCTX]
            kb = hin[:, NCTX : NCTX + NPAIR]
            ks = hin[:, NCTX + NPAIR : INS_COLS]

            psum = None
            for p in range(NPAIR):
                j = p % 64
                blk = p // 64
                d2 = dpool.tile([128, NCTX], F16, tag="d2")
                kcol = kb[:, p : p + 1]
                if _is_dve(p):
                    nc.vector.tensor_scalar(
                        d2[:], q2, kcol, 0.0, AL.subtract, AL.max
                    )
                    sel = selR
                else:
                    nc.scalar.activation(d2[:], q2, Abs, bias=kcol, scale=-1.0)
                    sel = selA
                if j == 0:
                    psum = ppool.tile([128, NCTX], F32, tag="acc")
                    nc.tensor.matmul(psum[:], selQ[:], q2, start=True, stop=False)
                nc.tensor.matmul(
                    psum[:],
                    sel[:, 126 - 2 * j : 254 - 2 * j],
                    d2[:],
                    start=False,
                    stop=(j == 63),
                )
                if j == 63:
                    stage = spool.tile([128, NCTX], F32, tag="stage")
                    nc.vector.tensor_scalar(
                        stage[:], psum[:], ks[:, blk : blk + 1], None, AL.add
                    )
                    nc.sync.dma_start(c[h, bass.ts(blk, 128), :], stage[:])


def build_nc(reps=1):
    # Bacc (not raw Bass): its compile() splits multi-sem sync waits into
    # event-semaphore instructions — TRN2 allows at most 1 wait per
    # instruction — and moves matmul waits onto ldweights.
    nc = bacc.Bacc("TRN2", target_bir_lowering=False, debug=False)
    ins = nc.dram_tensor("ins", [HPC, 2 * W, INS_COLS], F32, kind="ExternalInput").ap()
    c = nc.dram_tensor("c", [HPC, NCTX, NCTX], F32, kind="ExternalOutput").ap()
    with tile.TileContext(nc) as tc:
        _build_body(tc, c, ins, reps=reps)
    nc.compile()
    return nc


def _get_nc():
    global _NC_CACHE
    if _NC_CACHE is None:
        _NC_CACHE = build_nc()
    return _NC_CACHE


def run_on_hw(inputs_ins, reps=1, nc=None):
    """Run the per-core program (body repeated `reps` times) on all 8 cores."""
    if nc is None:
        nc = _get_nc() if reps == 1 else build_nc(reps)
    in_maps = [
        {"ins": inputs_ins[HPC * i : HPC * (i + 1)]} for i in range(N_CORES)
    ]
    return nc, run_bass_kernel_spmd(nc, in_maps, list(range(N_CORES)))


def host_prep(q, k):
    """Full q,k [2,512,8,64] -> packed per-head [128, 772] input blocks."""
    # [b, t, h, w] -> [(b h), t, w]
    qs = q.transpose(0, 2, 1, 3).reshape(BS * NH, NCTX, W)
    ks = k.transpose(0, 2, 1, 3).reshape(BS * NH, NCTX, W)
    qT = qs.transpose(0, 2, 1)  # [(b h), w, t]
    kT = ks.transpose(0, 2, 1)  # [(b h), w, s]
    ins = np.zeros((BS * NH, 2 * W, INS_COLS), np.float32)
    ins[:, 0:W, 0:NCTX] = qT
    ins[:, W : 2 * W, 0:NCTX] = qT
    ins[:, 0:W, NCTX : NCTX + NPAIR] = kT[:, :, 0::2]
    ins[:, W : 2 * W, NCTX : NCTX + NPAIR] = kT[:, :, 1::2]
    # -K_s/8 on VectorE rows (s = 2p+g with p on the VectorE path), else 0
    ksum = ks.sum(-1, dtype=np.float64).astype(np.float32)  # [(b h), s]
    dve_row = np.array(
        [_is_dve(s // 2) for s in range(NCTX)], np.float32
    )  # s -> row 2p+g keeps p = s//2
    kcorr = (-0.125 * ksum) * dve_row[None, :]  # [(b h), s]
    ins[:, :, NCTX + NPAIR :] = kcorr.reshape(BS * NH, NBLK, 128).transpose(0, 2, 1)
    return ins


def kernel(q, k):
    global LAST_RUN
    q = np.asarray(q, dtype=np.float32)
    k = np.asarray(k, dtype=np.float32)
    assert q.shape == (BS, NCTX, NH, W) and k.shape == (BS, NCTX, NH, W)

    ins = host_prep(q, k)
    in_maps = [{"ins": ins[HPC * i : HPC * (i + 1)]} for i in range(N_CORES)]

    nc = _get_nc()
    res = run_bass_kernel_spmd(nc, in_maps, list(range(N_CORES)))
    LAST_RUN = res
    outs = np.stack([res.results[i]["c"] for i in range(N_CORES)], axis=0)
    # [n_cores, HPC, s, t] -> [(b h), s, t] -> [b, h, s, t]
    return outs.reshape(BS, NH, NCTX, NCTX).astype(np.float32)


# revision 16
# speedup vs baseline: 1371.6679x; 1371.6679x over previous
"""L1-distance attention forward on 8 Trainium2 NeuronCores.

c[b,h,s,t] = -1/sqrt(64) * sum_w |q[b,t,h,w] - k[b,s,h,w]|

Full inputs q,k: [2, 512, 8, 64] f32. Output c: [2, 8, 512, 512] f32.

Sharding: the 16 (b,h) pairs are split 2-per-core across 8 cores (pure data
parallel, no collectives). Each core runs an identical single-core program.
The (cheap, [2,512,64]-sized) transposes of q/k are done host-side so the
device program is pure streaming compute.

Per-core DRAM input (pre-arranged on host), one [128, 772] block per head so a
single DMA feeds the whole head:
  ins[h, w + 64g, 0:512]    = q[t, w]          (both g halves identical)
  ins[h, w + 64g, 512+p]    = k[2p + g, w]
  ins[h, s_local, 768+blk]  = -K_s/8 on VectorE rows, 0 on ScalarE rows
                              (K_s = sum_w k[s, w]; s = 128*blk + s_local)

Per-core algorithm (per head), with s-pairs p = (s0=2p, s1=2p+1):
  - abs-diff tiles d2[w + 64g, t] (fp16, [128, 512]) on two engines:
      ScalarE pairs:  activation(Abs, in_=q2, scale=-1, bias=kb[:, p])
                      -> |q - k| directly
      VectorE pairs:  tensor_scalar(q2, kb[:, p], 0.0, subtract, max) @ 2x
                      -> relu(q - k), using |d| = 2 relu(d) - d and
                         sum_w d = Q_t - K_s handled by rank-1 corrections
  - TensorEngine reduces over w and accumulates 64 s-pairs into one
    [128, 512] PSUM block via sliding-window selector lhsT tiles
    (-1/8 for Abs pairs, -2/8 for relu pairs); one extra fp32 matmul per
    block adds Q_t/8 to VectorE rows using q2 itself as rhs
    (sum_partitions q2 = 2 Q_t, lhsT = 1/16 on VectorE columns).
    PSUM rows are then exactly c[s, t] rows, short of -K_s/8.
  - Staging tensor_scalar(psum + ks_col) -> SBUF adds the K_s correction,
    then DMA out (contiguous 256KB blocks).
"""

import os
from contextlib import ExitStack

import numpy as np

import concourse.bacc as bacc
import concourse.bass as bass
import concourse.mybir as mybir
import concourse.tile as tile
from concourse.bass_utils import run_bass_kernel_spmd

F32 = mybir.dt.float32
F16 = mybir.dt.float16

BS, NCTX, NH, W = 2, 512, 8, 64
N_CORES = 8
HPC = (BS * NH) // N_CORES  # heads (b,h pairs) per core = 2
NPAIR = NCTX // 2  # s-pairs per head = 256
NBLK = NCTX // 128  # 128-row output blocks per head = 4
INS_COLS = NCTX + NPAIR + NBLK  # 772

# s-pairs with (p % 16) < DVE_SPLIT go to VectorE (relu path), the rest to
# ScalarE (abs path). VectorE tensor_scalar runs ~2x the ScalarE rate.
DVE_SPLIT = 11


def _is_dve(p):
    return (p % 16) < DVE_SPLIT


_NC_CACHE = None
LAST_RUN = None  # BassKernelResults of the most recent run (for profiling)


def _build_body(tc, c, ins, reps=1, loop_iters=0):
    nc = tc.nc
    Abs = mybir.ActivationFunctionType.Abs
    AL = mybir.AluOpType
    with ExitStack() as ctx:
        if loop_iters:
            # timing mode: run the whole body loop_iters times on-device
            ctx.enter_context(tc.For_i(0, loop_iters, 1))
        const = ctx.enter_context(tc.tile_pool(name="const", bufs=1))
        prep = ctx.enter_context(tc.tile_pool(name="prep", bufs=2))
        dpool = ctx.enter_context(tc.tile_pool(name="d2", bufs=8))
        ppool = ctx.enter_context(tc.tile_pool(name="acc", bufs=2, space="PSUM"))
        spool = ctx.enter_context(tc.tile_pool(name="stage", bufs=2))

        # Sliding-window selectors: window [:, 126-2p : 254-2p] has its only
        # nonzero entries in column (126-2p)+m for m = 2p+g, summing partition
        # half g into psum row m. selA = -1/8 (abs pairs), selR = -2/8 (relu).
        selA = const.tile([128, 2 * 64 + 126], F16)
        nc.vector.memset(selA[:], 0.0)
        nc.vector.memset(selA[0:64, 126:127], -0.125)
        nc.vector.memset(selA[64:128, 127:128], -0.125)
        selR = const.tile([128, 2 * 64 + 126], F16)
        nc.vector.memset(selR[:], 0.0)
        nc.vector.memset(selR[0:64, 126:127], -0.25)
        nc.vector.memset(selR[64:128, 127:128], -0.25)
        # Q-correction selector: out[m, t] += (1/16) * sum_wg q2[wg, t] = Q_t/8
        # for VectorE rows m only (column zeroed for ScalarE rows).
        selQ = const.tile([128, 128], F32)
        nc.vector.memset(selQ[:], 1.0 / 16.0)
        for m0 in range(0, 128, 32):
            # rows m = 2p+g with p % 16 >= DVE_SPLIT -> cols [m0+2*DVE_SPLIT, m0+32)
            nc.vector.memset(selQ[:, m0 + 2 * DVE_SPLIT : m0 + 32], 0.0)

        for h in [h for _ in range(reps) for h in range(HPC)]:
            hin = prep.tile([128, INS_COLS], F32, tag="hin")
            nc.sync.dma_start(hin[:], ins[h])
            q2 = hin[:, 0:NCTX]
            kb = hin[:, NCTX : NCTX + NPAIR]
            ks = hin[:, NCTX + NPAIR : INS_COLS]

            psum = None
            for p in range(NPAIR):
                j = p % 64
                blk = p // 64
                d2 = dpool.tile([128, NCTX], F16, tag="d2")
                kcol = kb[:, p : p + 1]
                if _is_dve(p):
                    nc.vector.tensor_scalar(
                        d2[:], q2, kcol, 0.0, AL.subtract, AL.max
                    )
                    sel = selR
                else:
                    nc.scalar.activation(d2[:], q2, Abs, bias=kcol, scale=-1.0)
                    sel = selA
                if j == 0:
                    psum = ppool.tile([128, NCTX], F32, tag="acc")
                    nc.tensor.matmul(psum[:], selQ[:], q2, start=True, stop=False)
                nc.tensor.matmul(
                    psum[:],
                    sel[:, 126 - 2 * j : 254 - 2 * j],
                    d2[:],
                    start=False,
                    stop=(j == 63),
                )
                if j == 63:
                    stage = spool.tile([128, NCTX], F32, tag="stage")
                    nc.vector.tensor_scalar(
                        stage[:], psum[:], ks[:, blk : blk + 1], None, AL.add
                    )
                    nc.sync.dma_start(c[h, bass.ts(blk, 128), :], stage[:])


def build_nc(reps=1, loop_iters=0):
    # Bacc (not raw Bass): its compile() splits multi-sem sync waits into
    # event-semaphore instructions — TRN2 allows at most 1 wait per
    # instruction — and moves matmul waits onto ldweights.
    nc = bacc.Bacc("TRN2", target_bir_lowering=False, debug=False)
    ins = nc.dram_tensor("ins", [HPC, 2 * W, INS_COLS], F32, kind="ExternalInput").ap()
    c = nc.dram_tensor("c", [HPC, NCTX, NCTX], F32, kind="ExternalOutput").ap()
    with tile.TileContext(nc) as tc:
        _build_body(tc, c, ins, reps=reps, loop_iters=loop_iters)
    nc.compile()
    return nc


def _get_nc():
    global _NC_CACHE
    if _NC_CACHE is None:
        _NC_CACHE = build_nc()
    return _NC_CACHE


def run_on_hw(inputs_ins, reps=1, nc=None):
    """Run the per-core program (body repeated `reps` times) on all 8 cores."""
    if nc is None:
        nc = _get_nc() if reps == 1 else build_nc(reps)
    in_maps = [
        {"ins": inputs_ins[HPC * i : HPC * (i + 1)]} for i in range(N_CORES)
    ]
    return nc, run_bass_kernel_spmd(nc, in_maps, list(range(N_CORES)))


def host_prep(q, k):
    """Full q,k [2,512,8,64] -> packed per-head [128, 772] input blocks."""
    # [b, t, h, w] -> [(b h), t, w]
    qs = q.transpose(0, 2, 1, 3).reshape(BS * NH, NCTX, W)
    ks = k.transpose(0, 2, 1, 3).reshape(BS * NH, NCTX, W)
    qT = qs.transpose(0, 2, 1)  # [(b h), w, t]
    kT = ks.transpose(0, 2, 1)  # [(b h), w, s]
    ins = np.zeros((BS * NH, 2 * W, INS_COLS), np.float32)
    ins[:, 0:W, 0:NCTX] = qT
    ins[:, W : 2 * W, 0:NCTX] = qT
    ins[:, 0:W, NCTX : NCTX + NPAIR] = kT[:, :, 0::2]
    ins[:, W : 2 * W, NCTX : NCTX + NPAIR] = kT[:, :, 1::2]
    # -K_s/8 on VectorE rows (s = 2p+g with p on the VectorE path), else 0
    ksum = ks.sum(-1, dtype=np.float64).astype(np.float32)  # [(b h), s]
    dve_row = np.array(
        [_is_dve(s // 2) for s in range(NCTX)], np.float32
    )  # s -> row 2p+g keeps p = s//2
    kcorr = (-0.125 * ksum) * dve_row[None, :]  # [(b h), s]
    ins[:, :, NCTX + NPAIR :] = kcorr.reshape(BS * NH, NBLK, 128).transpose(0, 2, 1)
    return ins


def kernel(q, k):
    global LAST_RUN
    q = np.asarray(q, dtype=np.float32)
    k = np.asarray(k, dtype=np.float32)
    assert q.shape == (BS, NCTX, NH, W) and k.shape == (BS, NCTX, NH, W)

    ins = host_prep(q, k)
    in_maps = [{"ins": ins[HPC * i : HPC * (i + 1)]} for i in range(N_CORES)]

    nc = _get_nc()
    res = run_bass_kernel_spmd(nc, in_maps, list(range(N_CORES)))
    LAST_RUN = res
    outs = np.stack([res.results[i]["c"] for i in range(N_CORES)], axis=0)
    # [n_cores, HPC, s, t] -> [(b h), s, t] -> [b, h, s, t]
    return outs.reshape(BS, NH, NCTX, NCTX).astype(np.float32)
